# revision 1
# baseline (speedup 1.0000x reference)
"""Multi-head attention kernel for Trainium2, 8 NeuronCores.

Sharding: data-parallel over (batch, query-half): core i handles batch i//2
and query rows (i%2)*1024 ... +1024. Each core computes K/V over the full
sequence of its batch (K/V projection duplicated between the 2 cores of a
batch; no collectives), then attention for all 16 heads over its query half,
then the output projection for its query rows.

Per-core dataflow (activations kept "transposed" so the contraction dim sits
on SBUF partitions):
  xT  [1024, 2048]   d_model-major input; this core's query-half columns first
  V   = x @ Wv + bv          -> DRAM scratch [2048, 1024]
  K^T = Wk^T x^T (per pair)  -> DRAM scratch [8][128, 2048]
  Q^T = Wq^T x^T (per pair)  -> DRAM scratch [8][128, 1024]
  per head: scores^T[sk,sq] = K^T-slice.T @ Q^T ; P^T = exp(scores^T/8) (ACT)
            out^T[65, sq] = [V_h | 1].T @ P^T   (row 64 = softmax denominator)
            out^T[0:64] * (1/row64) -> outT
  y = outT.T @ Wo^T + bo  -> [1024, 1024] query-half output rows

All matmul operands are float32r (TF32-like, full PE rate at N>=256);
accumulation is fp32 in PSUM.
"""

import os

os.environ.setdefault("MYCRO_LOCAL_CACHE", "1")

import numpy as np

_B = lambda k, d: int(os.environ.get(k, d))

try:
    import concourse.bass as bass
except ImportError:  # pragma: no cover
    import sys

    for p in ("/opt/trn_rl_repo", "/root/.axon_site/_ro/trn_rl_repo"):
        if os.path.isdir(p) and p not in sys.path:
            sys.path.insert(0, p)
    import concourse.bass as bass

import concourse.mybir as mybir
import concourse.tile as tile
from concourse import bacc, bass_utils

F32R = mybir.dt.float32r
F32 = mybir.dt.float32
AF = mybir.ActivationFunctionType

B = 4
S = 2048
D_MODEL = 1024
H = 16
HD = 64
NPAIR = 8
KT = 8
SQ = 1024
NT = S // 128  # 16 sk-tiles
N_CORES = 8

_CACHE: dict = {}


def build_program():
    nc = bacc.Bacc("TRN2", target_bir_lowering=False, debug=False)

    xT = nc.dram_tensor("xT", [D_MODEL, S], F32R, kind="ExternalInput")
    wq = nc.dram_tensor("wq", [D_MODEL, D_MODEL], F32R, kind="ExternalInput")
    wk = nc.dram_tensor("wk", [D_MODEL, D_MODEL], F32R, kind="ExternalInput")
    wv = nc.dram_tensor("wv", [D_MODEL, D_MODEL], F32R, kind="ExternalInput")
    bv = nc.dram_tensor("bv", [1, D_MODEL], F32R, kind="ExternalInput")
    bq = nc.dram_tensor("bq", [NPAIR, 128], F32, kind="ExternalInput")
    bk = nc.dram_tensor("bk", [NPAIR, 128], F32, kind="ExternalInput")
    wo = nc.dram_tensor("wo", [D_MODEL, D_MODEL], F32R, kind="ExternalInput")
    bo = nc.dram_tensor("bo", [1, D_MODEL], F32R, kind="ExternalInput")
    ones_in = nc.dram_tensor("ones_in", [1, 128], F32R, kind="ExternalInput")
    ones_tk = nc.dram_tensor("ones_tk", [128, NT], F32R, kind="ExternalInput")
    y = nc.dram_tensor("y", [SQ, D_MODEL], F32, kind="ExternalOutput")

    with tile.TileContext(nc) as tc:
        with (
            tc.tile_pool(name="pers", bufs=1) as pers,
            tc.tile_pool(name="dram", bufs=1, space="DRAM") as dram,
        ):
            outT = pers.tile([128, NPAIR, SQ], F32R)  # normalized out^T, 4 MiB
            ones_sb = pers.tile([1, 128], F32R)
            bv_sb = pers.tile([1, D_MODEL], F32R)
            bo_sb = pers.tile([1, D_MODEL], F32R)
            bq_sb = pers.tile([128, NPAIR], F32)
            bk_sb = pers.tile([128, NPAIR], F32)
            nc.sync.dma_start(ones_sb[:], ones_in.ap())
            nc.sync.dma_start(bv_sb[:], bv.ap())
            nc.sync.dma_start(bo_sb[:], bo.ap())
            nc.sync.dma_start(bq_sb[:], bq.ap().rearrange("p r -> r p"))
            nc.sync.dma_start(bk_sb[:], bk.ap().rearrange("p r -> r p"))

            vscr = dram.tile([S, D_MODEL], F32R)
            qscr = dram.tile([NPAIR, 128, SQ], F32R)
            kscr = dram.tile([NPAIR, 128, S], F32R)

            # attention streaming pools (opened early so prefetches can be
            # emitted from inside the KQ phase)
            with (
                tc.tile_pool(name="vst", bufs=_B("VST", 3)) as vsp,
                tc.tile_pool(name="ktp", bufs=_B("KTP", 2)) as ktp,
                tc.tile_pool(name="qtp", bufs=_B("QTP", 2)) as qtp,
            ):
                vp_t, kt_t, qt_t = {}, {}, {}

                def prefetch_pair(p):
                    vp_sb = vsp.tile([128, NT, 130], F32R, tag="vp", name=f"vp{p}")
                    for a in range(2):
                        nc.sync.dma_start(
                            vp_sb[:, :, a * 65 : a * 65 + 64],
                            vscr[:, 128 * p + a * 64 : 128 * p + (a + 1) * 64].rearrange(
                                "(t r) c -> r t c", r=128
                            ),
                        )
                        nc.sync.dma_start(
                            vp_sb[:, :, a * 65 + 64 : a * 65 + 65],
                            ones_tk.ap().rearrange("r (t o) -> r t o", o=1),
                        )
                    vp_t[p] = vp_sb
                    kt_sb = ktp.tile([128, S], F32R, tag="kt", name=f"kt{p}")
                    nc.sync.dma_start(kt_sb[:], kscr[p])
                    kt_t[p] = kt_sb
                    qt_sb = qtp.tile([128, SQ], F32R, tag="qt", name=f"qt{p}")
                    nc.sync.dma_start(qt_sb[:], qscr[p])
                    qt_t[p] = qt_sb

                # ---------------- phase V + KQ (xT resident) ----------------
                with tc.tile_pool(name="xp", bufs=1) as xp:
                    xt_sb = xp.tile([128, KT, S], F32R)
                    with (
                        tc.tile_pool(name="vw", bufs=2) as vwp,
                        tc.tile_pool(name="vps", bufs=_B("VPS", 5), space="PSUM") as vpsp,
                        tc.tile_pool(name="vd", bufs=_B("VD", 5)) as vdp,
                    ):
                        # first V weight block, then xT k-tiles (so compute can
                        # start as soon as the needed slices land)
                        wv_sbs = []
                        for i, c0 in enumerate((0, 512)):
                            wv_sb = vwp.tile([128, KT, 512], F32R, tag="wv", name=f"wv{c0}")
                            # per-k DMAs so the first matmul's operands land
                            # after ~1.3 MiB instead of the full 3 MiB
                            for k in range(KT):
                                nc.sync.dma_start(
                                    wv_sb[:, k, :],
                                    wv.ap()[128 * k : 128 * (k + 1), c0 : c0 + 512],
                                )
                                if i == 0:
                                    nc.sync.dma_start(
                                        xt_sb[:, k, :],
                                        xT.ap()[128 * k : 128 * (k + 1), :],
                                    )
                            wv_sbs.append(wv_sb)
                        for i, c0 in enumerate((0, 512)):
                            wv_sb = wv_sbs[i]
                            for t in range(NT):
                                vps = vpsp.tile(
                                    [128, 512], F32, tag="vps", name=f"vps{c0}_{t}"
                                )
                                for k in range(KT):
                                    nc.tensor.matmul(
                                        vps[:],
                                        xt_sb[:, k, t * 128 : (t + 1) * 128],
                                        wv_sb[:, k, :],
                                        start=(k == 0),
                                        stop=False,
                                    )
                                nc.tensor.matmul(
                                    vps[:],
                                    ones_sb[:],
                                    bv_sb[:, c0 : c0 + 512],
                                    start=False,
                                    stop=True,
                                )
                                vsb = vdp.tile([128, 512], F32R, tag="vsb", name=f"vsb{c0}_{t}")
                                nc.scalar.activation(vsb[:], vps[:], AF.Copy)
                                nc.sync.dma_start(
                                    vscr[t * 128 : (t + 1) * 128, c0 : c0 + 512], vsb[:]
                                )

                    # K^T and Q^T per head pair -> DRAM scratch
                    with (
                        tc.tile_pool(name="wkq", bufs=_B("WKQ", 2)) as wkqp,
                        tc.tile_pool(name="kps", bufs=_B("KPS", 5), space="PSUM") as kpsp,
                        tc.tile_pool(name="qps", bufs=_B("QPS", 3), space="PSUM") as qpsp,
                        tc.tile_pool(name="kd", bufs=2) as kdp,
                        tc.tile_pool(name="qd", bufs=2) as qdp,
                    ):
                        for p in range(NPAIR):
                            wk_sb = wkqp.tile([128, KT, 128], F32R, tag="wk", name=f"wk{p}")
                            nc.sync.dma_start(
                                wk_sb[:],
                                wk.ap()[:, 128 * p : 128 * (p + 1)].rearrange(
                                    "(k r) c -> r k c", r=128
                                ),
                            )
                            wq_sb = wkqp.tile([128, KT, 128], F32R, tag="wq", name=f"wq{p}")
                            nc.sync.dma_start(
                                wq_sb[:],
                                wq.ap()[:, 128 * p : 128 * (p + 1)].rearrange(
                                    "(k r) c -> r k c", r=128
                                ),
                            )
                            kps = [
                                kpsp.tile([128, 512], F32, tag="kps", name=f"kps{p}_{j}")
                                for j in range(4)
                            ]
                            qps = [
                                qpsp.tile([128, 512], F32, tag="qps", name=f"qps{p}_{j}")
                                for j in range(2)
                            ]
                            for k in range(KT):
                                for j in range(4):
                                    nc.tensor.matmul(
                                        kps[j][:],
                                        wk_sb[:, k, :],
                                        xt_sb[:, k, j * 512 : (j + 1) * 512],
                                        start=(k == 0),
                                        stop=(k == KT - 1),
                                    )
                                for j in range(2):
                                    nc.tensor.matmul(
                                        qps[j][:],
                                        wq_sb[:, k, :],
                                        xt_sb[:, k, j * 512 : (j + 1) * 512],
                                        start=(k == 0),
                                        stop=(k == KT - 1),
                                    )
                            ksb = kdp.tile([128, S], F32R, tag="ksb", name=f"ksb{p}")
                            for j in range(4):
                                nc.vector.tensor_scalar_add(
                                    ksb[:, j * 512 : (j + 1) * 512],
                                    kps[j][:],
                                    bk_sb[:, p : p + 1],
                                )
                            nc.sync.dma_start(kscr[p], ksb[:])
                            qsb = qdp.tile([128, SQ], F32R, tag="qsb", name=f"qsb{p}")
                            for j in range(2):
                                nc.vector.tensor_scalar_add(
                                    qsb[:, j * 512 : (j + 1) * 512],
                                    qps[j][:],
                                    bq_sb[:, p : p + 1],
                                )
                            nc.sync.dma_start(qscr[p], qsb[:])
                            if p == NPAIR - 1:
                                prefetch_pair(0)

                # ---------------- attention + wo prefetch ----------------
                with tc.tile_pool(name="wop", bufs=1) as wop:
                    wo_sb = wop.tile([128, KT, D_MODEL], F32R)
                    nc.sync.dma_start(
                        wo_sb[:], wo.ap().rearrange("(k r) c -> r k c", r=128)
                    )
                    attn_pools = (
                        tc.tile_pool(name="pt", bufs=_B("PT", 5)),
                        tc.tile_pool(name="scp", bufs=2, space="PSUM"),
                        tc.tile_pool(name="avp", bufs=1, space="PSUM"),
                        tc.tile_pool(name="asb", bufs=2),
                        tc.tile_pool(name="sm", bufs=2),
                        tc.tile_pool(name="ntp", bufs=2),
                    )
                    import contextlib
                    _stk = contextlib.ExitStack()
                    ptp, scp, avp, asbp, smp, ntp = (_stk.enter_context(pl) for pl in attn_pools)
                    # attention with one-chunk-delayed attn@V emission that
                    # also crosses head/pair boundaries, so the PE never waits
                    # on the exp of the chunk it just issued.
                    pend = None  # (pt, grp, vp_sb, a, av, is_last, p)

                    def flush_pend():
                        pt_, grp_, vps_, a_, av_, last_, p_ = pend
                        for gi, (t, j) in enumerate(grp_):
                            nc.tensor.matmul(
                                av_[:, j * 512 : (j + 1) * 512],
                                vps_[:, t, a_ * 65 : (a_ + 1) * 65],
                                pt_[:, gi * 512 : (gi + 1) * 512],
                                start=(t == 0),
                                stop=(t == NT - 1),
                            )
                        if last_:
                            avsb = asbp.tile(
                                [65, SQ], F32, tag="avsb", name=f"avsb{p_}_{a_}"
                            )
                            nc.vector.tensor_copy(avsb[:], av_[:])
                            rc = smp.tile([128, SQ], F32, tag="rc", name=f"rc{p_}_{a_}")
                            nc.vector.reciprocal(rc[64:65, :], avsb[64:65, :])
                            rz = smp.tile([1, SQ], F32, tag="rz", name=f"rz{p_}_{a_}")
                            nc.sync.dma_start(rz[:], rc[64:65, :])
                            bc = smp.tile([64, SQ], F32, tag="bc", name=f"bc{p_}_{a_}")
                            nc.gpsimd.partition_broadcast(bc[:], rz[:])
                            nt = ntp.tile([64, SQ], F32R, tag="nt", name=f"nt{p_}_{a_}")
                            nc.vector.tensor_mul(nt[:], avsb[0:64, :], bc[:])
                            nc.sync.dma_start(
                                outT[a_ * 64 : (a_ + 1) * 64, p_, :], nt[:]
                            )

                    for p in range(NPAIR):
                        if p + 1 < NPAIR:
                            prefetch_pair(p + 1)
                        vp_sb, kt_sb, qt_sb = vp_t.pop(p), kt_t.pop(p), qt_t.pop(p)
                        for a in range(2):
                            # single-buffered av: drained to SBUF right after
                            # the last accumulation so the psum slot frees fast
                            av = avp.tile([65, SQ], F32, tag="av", name=f"av{p}_{a}")
                            # scores/exp in 1536-wide chunks (3 psum banks x2):
                            # 3 (t, sq-half) units of 512 columns per chunk;
                            # partition meaning (sk-tile) varies per column
                            # range, which is fine for elementwise exp.
                            units = [(t, j) for t in range(NT) for j in range(2)]
                            # alternate which end of the head carries the short
                            # 2-unit chunk, so short chunks of adjacent heads
                            # meet at every other head boundary
                            short_first = (p * 2 + a) % 2 == 1
                            ci = 0
                            while units:
                                n = 2 if (ci == 0 and short_first) else 3
                                grp, units = units[:n], units[n:]
                                w = 512 * len(grp)
                                sc = scp.tile(
                                    [128, w], F32, tag="sc", name=f"sc{p}_{a}_{ci}"
                                )
                                for gi, (t, j) in enumerate(grp):
                                    nc.tensor.matmul(
                                        sc[:, gi * 512 : (gi + 1) * 512],
                                        kt_sb[a * 64 : (a + 1) * 64, t * 128 : (t + 1) * 128],
                                        qt_sb[a * 64 : (a + 1) * 64, j * 512 : (j + 1) * 512],
                                        start=True,
                                        stop=True,
                                    )
                                pt = ptp.tile(
                                    [128, w], F32R, tag="pt", name=f"pt{p}_{a}_{ci}"
                                )
                                nc.scalar.activation(pt[:], sc[:], AF.Exp, scale=0.125)
                                if pend is not None:
                                    flush_pend()
                                pend = (pt, grp, vp_sb, a, av, not units, p)
                                ci += 1
                    flush_pend()
                    pend = None

                    _stk.close()

                    # ---------------- output projection ----------------
                    with (
                        tc.tile_pool(name="yps", bufs=_B("YPS", 8), space="PSUM") as ypsp,
                        tc.tile_pool(name="yd", bufs=_B("YD", 6)) as ydp,
                    ):
                        for m in range(SQ // 128):
                            yps = [
                                ypsp.tile([128, 512], F32, tag="yp", name=f"yp{m}_{nb}")
                                for nb in range(2)
                            ]
                            for k in range(KT):
                                for nb in range(2):
                                    # nb-inner: both column blocks reuse the
                                    # same stationary outT slice back-to-back
                                    nc.tensor.matmul(
                                        yps[nb][:],
                                        outT[:, k, m * 128 : (m + 1) * 128],
                                        wo_sb[:, k, nb * 512 : (nb + 1) * 512],
                                        start=(k == 0),
                                        stop=False,
                                    )
                            for nb in range(2):
                                nc.tensor.matmul(
                                    yps[nb][:],
                                    ones_sb[:],
                                    bo_sb[:, nb * 512 : (nb + 1) * 512],
                                    start=False,
                                    stop=True,
                                )
                                ysb = ydp.tile(
                                    [128, 512], F32, tag="ysb", name=f"ysb{m}_{nb}"
                                )
                                nc.vector.tensor_copy(ysb[:], yps[nb][:])
                                nc.sync.dma_start(
                                    y.ap()[
                                        m * 128 : (m + 1) * 128,
                                        nb * 512 : (nb + 1) * 512,
                                    ],
                                    ysb[:],
                                )

    nc.compile()
    return nc


def prep_inputs(x, Wq, bq, Wk, bk, Wv, bv, Wo, bo):
    """Host-side sharding: returns per-core input maps (numpy only)."""
    x = np.asarray(x, dtype=np.float32)
    Wq = np.asarray(Wq, dtype=np.float32)
    Wk = np.asarray(Wk, dtype=np.float32)
    Wv = np.asarray(Wv, dtype=np.float32)
    Wo = np.asarray(Wo, dtype=np.float32)
    bq = np.asarray(bq, dtype=np.float32)
    bk = np.asarray(bk, dtype=np.float32)
    bv = np.asarray(bv, dtype=np.float32)
    bo = np.asarray(bo, dtype=np.float32)

    shared = {
        "wq": np.ascontiguousarray(Wq.transpose(1, 0, 2).reshape(D_MODEL, D_MODEL)),
        "wk": np.ascontiguousarray(Wk.transpose(1, 0, 2).reshape(D_MODEL, D_MODEL)),
        "wv": np.ascontiguousarray(Wv.transpose(1, 0, 2).reshape(D_MODEL, D_MODEL)),
        "bv": bv.reshape(1, D_MODEL).copy(),
        "bq": np.ascontiguousarray(bq.reshape(NPAIR, 128)),
        "bk": np.ascontiguousarray(bk.reshape(NPAIR, 128)),
        "wo": np.ascontiguousarray(Wo.T),
        "bo": bo.reshape(1, D_MODEL).copy(),
        "ones_in": np.ones((1, 128), dtype=np.float32),
        "ones_tk": np.ones((128, NT), dtype=np.float32),
    }
    in_maps = []
    for core in range(N_CORES):
        b, half = divmod(core, 2)
        xt = x[b].T
        if half == 0:
            xt_core = xt
        else:
            xt_core = np.concatenate([xt[:, SQ:], xt[:, :SQ]], axis=1)
        in_maps.append({"xT": np.ascontiguousarray(xt_core), **shared})
    return in_maps


def assemble_output(results):
    y = np.empty((B, S, D_MODEL), dtype=np.float32)
    for core in range(N_CORES):
        b, half = divmod(core, 2)
        y[b, half * SQ : (half + 1) * SQ, :] = results[core]["y"]
    return y


def _get_runner():
    """Build the program + jitted 8-core executor once; reuse across calls."""
    if "runner" in _CACHE:
        return _CACHE["runner"]

    import jax
    import concourse.mybir as mb
    from concourse import bass2jax
    from jax.sharding import Mesh, PartitionSpec
    from jax.experimental.shard_map import shard_map

    nc = build_program()
    _CACHE["nc"] = nc
    bass2jax.install_neuronx_cc_hook()

    partition_name = (
        nc.partition_id_tensor.name if nc.partition_id_tensor is not None else None
    )
    in_names, out_names, out_avals = [], [], []
    for alloc in nc.m.functions[0].allocations:
        if not isinstance(alloc, mb.MemoryLocationSet):
            continue
        name = alloc.memorylocations[0].name
        if alloc.kind == "ExternalInput":
            if name != partition_name:
                in_names.append(name)
        elif alloc.kind == "ExternalOutput":
            out_names.append(name)
            out_avals.append(
                jax.core.ShapedArray(tuple(alloc.tensor_shape), mb.dt.np(alloc.dtype))
            )
    n_params = len(in_names)
    n_outs = len(out_avals)
    all_in_names = in_names + out_names
    if partition_name is not None:
        all_in_names = all_in_names + [partition_name]

    def _body(*args):
        operands = list(args)
        if partition_name is not None:
            operands.append(bass2jax.partition_id_tensor())
        outs = bass2jax._bass_exec_p.bind(
            *operands,
            out_avals=tuple(out_avals),
            in_names=tuple(all_in_names),
            out_names=tuple(out_names),
            lowering_input_output_aliases=(),
            sim_require_finite=True,
            sim_require_nnan=True,
            nc=nc,
        )
        return tuple(outs)

    devices = jax.devices()[:N_CORES]
    mesh = Mesh(np.asarray(devices), ("core",))
    donate = tuple(range(n_params, n_params + n_outs))
    sharded = jax.jit(
        shard_map(
            _body,
            mesh=mesh,
            in_specs=(PartitionSpec("core"),) * (n_params + n_outs),
            out_specs=(PartitionSpec("core"),) * n_outs,
            check_rep=False,
        ),
        donate_argnums=donate,
        keep_unused=True,
    )

    import hashlib

    from jax.sharding import NamedSharding

    sharding = NamedSharding(mesh, PartitionSpec("core"))
    dev_cache: dict = {}

    # donated output buffers are created on-device (no host->device transfer)
    import jax.numpy as jnp

    zeros_fns = [
        jax.jit(
            (lambda shape, dtype: (lambda: jnp.zeros(shape, dtype)))(
                (N_CORES * a.shape[0], *a.shape[1:]), a.dtype
            ),
            out_shardings=sharding,
        )
        for a in out_avals
    ]

    def _dev_input(nm, in_maps):
        arrs = [np.asarray(m[nm]) for m in in_maps]
        h = hashlib.blake2b(digest_size=16)
        for a in arrs:
            h.update(a.tobytes())
        key = (nm, h.hexdigest())
        if key not in dev_cache:
            if len(dev_cache) > 64:
                dev_cache.clear()
            dev_cache[key] = jax.device_put(
                np.concatenate(arrs, axis=0), sharding
            )
        return dev_cache[key]

    def run(in_maps):
        concat_in = [_dev_input(nm, in_maps) for nm in in_names]
        concat_zeros = [zf() for zf in zeros_fns]
        out_arrs = sharded(*concat_in, *concat_zeros)
        return [
            {
                nm: np.asarray(out_arrs[i]).reshape(N_CORES, *out_avals[i].shape)[c]
                for i, nm in enumerate(out_names)
            }
            for c in range(N_CORES)
        ]

    _CACHE["runner"] = run
    return run


def kernel(**inputs):
    run = _get_runner()
    in_maps = prep_inputs(**inputs)
    return assemble_output(run(in_maps))



# revision 12
# speedup vs baseline: 1.3904x; 1.3904x over previous
"""Multi-head attention kernel for Trainium2, 8 NeuronCores.

Sharding: data-parallel over (batch, query-half): core i handles batch i//2
and query rows (i%2)*1024 ... +1024. Each core computes K/V over the full
sequence of its batch, Q over its query half, attention for all 16 heads,
and the output projection for its query rows. No collectives.

Fully fused, SBUF-resident pipeline (no DRAM scratch):
  per head-pair p (2 heads):
    K^T = (8*Wk_p)^T x^T + 8bk -> fp8 e4m3 [128, 2048]   (PE fp32r + DVE drain)
    Q^T = (8*Wq_p)^T x^T + 8bq -> fp8 [128, 1024]
    V   = x Wv_duo             -> bf16 [128 sk, 16t, 260] (pair-duo, ones cols)
    scores^T[sk,sq] = 2*K^T_slice.T Q^T  via fp8 DoubleRow matmul (both
        operands stride-0-doubled; x2 folded into the exp scale) -> PSUM
    P^T = exp(scores/1024) -> bf16 (ACT, 1024-col chunks)
    AV flipped: out[sq,65] = sum_t P^T-tile.T @ [V|1]  (bf16, all 128 output
        partitions used; col 64 = softmax denominator)
    normalize on DVE (per-partition reciprocal; no cross-partition broadcast)
    PE-transpose out -> outT[d, sq] bf16 (+bv bias on the DVE drain)
  y = outT^T Wo^T + bo (fp32r moving, bf16 stationary)

Engine budget per core (cost model): PE ~284us, ACT exp ~266us, DVE ~165us,
DMA ~28 MiB. fp8 scores rel-err ~9e-3 vs fp32 reference (2e-2 budget).
"""

import os

os.environ.setdefault("MYCRO_LOCAL_CACHE", "1")

import numpy as np

_B = lambda k, d: int(os.environ.get(k, d))

try:
    import concourse.bass as bass
except ImportError:  # pragma: no cover
    import sys

    for p in ("/opt/trn_rl_repo", "/root/.axon_site/_ro/trn_rl_repo"):
        if os.path.isdir(p) and p not in sys.path:
            sys.path.insert(0, p)
    import concourse.bass as bass

import concourse.mybir as mybir
import concourse.tile as tile
from concourse import bacc, bass_utils

F32R = mybir.dt.float32r
F32 = mybir.dt.float32
BF16 = mybir.dt.bfloat16
FP8 = mybir.dt.float8e4
AF = mybir.ActivationFunctionType
DR = mybir.MatmulPerfMode.DoubleRow

B = 4
S = 2048
D_MODEL = 1024
H = 16
HD = 64
NPAIR = 8
KT = 8
SQ = 1024
NT = S // 128  # 16 sk-tiles
N_CORES = 8
EXP_SCALE = 1.0 / 1024.0  # 1/8 softmax scale / (8*8 fp8 scales) / 2 (stride-0 DR)

_CACHE: dict = {}


def build_program():
    nc = bacc.Bacc("TRN2", target_bir_lowering=False, debug=False)

    xT = nc.dram_tensor("xT", [D_MODEL, S], F32R, kind="ExternalInput")
    wq = nc.dram_tensor("wq", [D_MODEL, D_MODEL], F32R, kind="ExternalInput")
    wk = nc.dram_tensor("wk", [D_MODEL, D_MODEL], F32R, kind="ExternalInput")
    wv = nc.dram_tensor("wv", [D_MODEL, D_MODEL], F32R, kind="ExternalInput")
    wo = nc.dram_tensor("wo", [D_MODEL, D_MODEL], BF16, kind="ExternalInput")
    bq = nc.dram_tensor("bq", [128, NPAIR], F32, kind="ExternalInput")
    bk = nc.dram_tensor("bk", [128, NPAIR], F32, kind="ExternalInput")
    bv = nc.dram_tensor("bv", [128, NPAIR], F32, kind="ExternalInput")
    bo = nc.dram_tensor("bo", [1, D_MODEL], F32R, kind="ExternalInput")
    ones_in = nc.dram_tensor("ones_in", [1, 128], F32R, kind="ExternalInput")
    ident_in = nc.dram_tensor("ident_in", [128, 128], BF16, kind="ExternalInput")
    y = nc.dram_tensor("y", [SQ, D_MODEL], F32, kind="ExternalOutput")
    dbg = os.environ.get("KDBG", "0") == "1"
    if dbg:
        dbg_kt = nc.dram_tensor("dbg_kt", [128, S], F32, kind="ExternalOutput")
        dbg_qt = nc.dram_tensor("dbg_qt", [128, SQ], F32, kind="ExternalOutput")
        dbg_vt = nc.dram_tensor("dbg_vt", [128, NT, 260], F32, kind="ExternalOutput")
        dbg_ot = nc.dram_tensor("dbg_ot", [128, NPAIR, SQ], F32, kind="ExternalOutput")

    with tile.TileContext(nc) as tc:
        with tc.tile_pool(name="pers", bufs=1) as pers:
            xt_sb = pers.tile([128, KT, S], F32R)
            outT = pers.tile([128, NPAIR, SQ], BF16)
            wo_sb = pers.tile([128, KT, D_MODEL], BF16)
            ident_sb = pers.tile([128, 128], BF16)
            ones_sb = pers.tile([1, 128], F32R)
            bq_sb = pers.tile([128, NPAIR], F32)
            bk_sb = pers.tile([128, NPAIR], F32)
            bv_sb = pers.tile([128, NPAIR], F32)
            bo_sb = pers.tile([1, D_MODEL], F32R)
            nc.sync.dma_start(ident_sb[:], ident_in.ap())
            nc.sync.dma_start(ones_sb[:], ones_in.ap())
            nc.sync.dma_start(bq_sb[:], bq.ap())
            nc.sync.dma_start(bk_sb[:], bk.ap())
            nc.sync.dma_start(bv_sb[:], bv.ap())
            nc.sync.dma_start(bo_sb[:], bo.ap())

            def dma_x(j):
                # per (seq-block, kc) so the first K chunks can start early
                for kc in range(KT):
                    nc.sync.dma_start(
                        xt_sb[:, kc, j * 512 : (j + 1) * 512],
                        xT.ap()[kc * 128 : (kc + 1) * 128, j * 512 : (j + 1) * 512],
                    )

            with (
                tc.tile_pool(name="wkq", bufs=2) as wkqp,
                tc.tile_pool(name="wvd", bufs=2) as wvdp,
                tc.tile_pool(name="ktp", bufs=2) as ktp,
                tc.tile_pool(name="qtp", bufs=2) as qtp,
                tc.tile_pool(name="vtp", bufs=2) as vtp,
                tc.tile_pool(name="ptp", bufs=_B("PTP", 11)) as ptp,
                tc.tile_pool(name="ntp", bufs=2) as ntp,
                tc.tile_pool(name="scp", bufs=_B("SCP", 2), space="PSUM") as scp,
                tc.tile_pool(name="ppp", bufs=_B("PPP", 2), space="PSUM") as ppp,
                tc.tile_pool(name="avt", bufs=_B("AVT", 2), space="PSUM") as avtp,
            ):
                wk_t, wq_t, wv_t = {}, {}, {}
                kt_t, qt_t, vt_t = {}, {}, {}

                def dma_wkq(p):
                    wk_sb = wkqp.tile([128, KT, 128], F32R, tag="wk", name=f"wk{p}")
                    nc.sync.dma_start(
                        wk_sb[:],
                        wk.ap()[:, 128 * p : 128 * (p + 1)].rearrange(
                            "(k r) c -> r k c", r=128
                        ),
                    )
                    wk_t[p] = wk_sb
                    wq_sb = wkqp.tile([128, KT, 128], F32R, tag="wq", name=f"wq{p}")
                    nc.sync.dma_start(
                        wq_sb[:],
                        wq.ap()[:, 128 * p : 128 * (p + 1)].rearrange(
                            "(k r) c -> r k c", r=128
                        ),
                    )
                    wq_t[p] = wq_sb

                def dma_wv(d):
                    wv_sb = wvdp.tile([128, KT, 256], F32R, tag="wv", name=f"wv{d}")
                    nc.sync.dma_start(
                        wv_sb[:],
                        wv.ap()[:, 256 * d : 256 * (d + 1)].rearrange(
                            "(k r) c -> r k c", r=128
                        ),
                    )
                    wv_t[d] = wv_sb

                def kq_closures(p):
                    """K (8) + Q (4) psum-chunk closures for pair p."""
                    kt_sb = ktp.tile([128, S], FP8, tag="kt", name=f"kt{p}")
                    qt_sb = qtp.tile([128, SQ], FP8, tag="qt", name=f"qt{p}")
                    kt_t[p], qt_t[p] = kt_sb, qt_sb
                    cls = []

                    def k_chunk(blk):
                        def f():
                            ps = ppp.tile([128, 256], F32, tag="pp", name=f"kp{p}_{blk}")
                            for kc in range(KT):
                                nc.tensor.matmul(
                                    ps[:],
                                    wk_t[p][:, kc, :],
                                    xt_sb[:, kc, blk * 256 : (blk + 1) * 256],
                                    start=(kc == 0),
                                    stop=(kc == KT - 1),
                                )
                            nc.vector.tensor_scalar_add(
                                kt_sb[:, blk * 256 : (blk + 1) * 256],
                                ps[:],
                                bk_sb[:, p : p + 1],
                            )
                        return f

                    def q_chunk(blk):
                        def f():
                            ps = ppp.tile([128, 256], F32, tag="pp", name=f"qp{p}_{blk}")
                            for kc in range(KT):
                                nc.tensor.matmul(
                                    ps[:],
                                    wq_t[p][:, kc, :],
                                    xt_sb[:, kc, blk * 256 : (blk + 1) * 256],
                                    start=(kc == 0),
                                    stop=(kc == KT - 1),
                                )
                            nc.vector.tensor_scalar_add(
                                qt_sb[:, blk * 256 : (blk + 1) * 256],
                                ps[:],
                                bq_sb[:, p : p + 1],
                            )
                        return f

                    # K first (scores of (a=0, j=0) touch all sk), Q interleaved
                    for blk in range(8):
                        cls.append(k_chunk(blk))
                        if blk < 4:
                            cls.append(q_chunk(blk))
                    return cls

                def v_closures(d):
                    """16 V psum-chunk closures for pair-duo d (pairs 2d, 2d+1)."""
                    vt_sb = vtp.tile([128, NT, 260], BF16, tag="vt", name=f"vt{d}")
                    vt_t[d] = vt_sb
                    cls = []

                    def ones_cols():
                        nc.vector.memset(
                            vt_sb[:].rearrange("p t (c f) -> p t c f", f=65)[
                                :, :, :, 64:65
                            ],
                            1.0,
                        )

                    cls.append(ones_cols)

                    def v_chunk(t):
                        def f():
                            ps = ppp.tile([128, 256], F32, tag="pp", name=f"vp{d}_{t}")
                            for kc in range(KT):
                                nc.tensor.matmul(
                                    ps[:],
                                    xt_sb[:, kc, t * 128 : (t + 1) * 128],
                                    wv_t[d][:, kc, :],
                                    start=(kc == 0),
                                    stop=(kc == KT - 1),
                                )
                            nc.vector.tensor_copy(
                                vt_sb[:, t, :].rearrange("p (c f) -> p c f", f=65)[
                                    :, :, 0:64
                                ],
                                ps[:].rearrange("p (c f) -> p c f", f=64),
                            )
                        return f

                    for t in range(NT):
                        cls.append(v_chunk(t))
                    return cls

                # -------------------- attention pipeline --------------------
                # AV is emitted as a per-unit burst with each ms-group's
                # accumulation sequential: PSUM start_tensor_calc pends the
                # whole 2KB bank, so groups sharing a bank must not interleave.
                pend = [None]

                def finish_unit(p, a, j, av):
                    rc = ntp.tile([128, 4], F32, tag="rc", name=f"rc{p}_{a}_{j}")
                    nc.vector.reciprocal(rc[:], av[:, :, 64])
                    nt = ntp.tile([128, 4, 64], BF16, tag="nt", name=f"nt{p}_{a}_{j}")
                    nc.vector.tensor_mul(
                        nt[:], av[:, :, 0:64], rc[:].unsqueeze(2).broadcast_to([128, 4, 64])
                    )
                    tp = avtp.tile(
                        [128, 4, 128], BF16, tag="avtp", name=f"tp{p}_{a}_{j}"
                    )
                    for ms in range(4):
                        nc.tensor.transpose(
                            tp[a * 64 : (a + 1) * 64, ms, :], nt[:, ms, :], ident_sb[:]
                        )
                    nc.vector.tensor_scalar_add(
                        outT[a * 64 : (a + 1) * 64, p, j * 512 : (j + 1) * 512],
                        tp[a * 64 : (a + 1) * 64, :, :].rearrange("p c f -> p (c f)"),
                        bv_sb[a * 64 : (a + 1) * 64, p : p + 1],
                    )

                def flush_pend():
                    if pend[0] is None:
                        return
                    p_, a_, j_, av_, pts_ = pend[0]
                    pend[0] = None
                    pin = p_ % 2
                    voff = pin * 130 + a_ * 65
                    vt_sb = vt_t[p_ // 2]
                    for ms in range(4):
                        for c in range(8):
                            for ti in range(2):
                                t = 2 * c + ti
                                nc.tensor.matmul(
                                    av_[:, ms, :],
                                    pts_[c][:, ti, ms * 128 : (ms + 1) * 128],
                                    vt_sb[:, t, voff : voff + 65],
                                    start=(t == 0),
                                    stop=(t == NT - 1),
                                )
                    finish_unit(p_, a_, j_, av_)

                def attn_pair(p, closures):
                    n, taken, slot = len(closures), 0, 0
                    kt_sb, qt_sb = kt_t[p], qt_t[p]
                    for a in range(2):
                        for j in range(2):
                            av = avtp.tile(
                                [128, 4, 65], F32, tag="avtp", name=f"av{p}_{a}_{j}"
                            )
                            pts = []
                            for c in range(8):
                                sc = scp.tile(
                                    [128, 2, 512], F32, tag="sc", name=f"sc{p}_{a}_{j}_{c}"
                                )
                                for ti in range(2):
                                    t = 2 * c + ti
                                    lhs = (
                                        kt_sb[a * 64 : (a + 1) * 64, t * 128 : (t + 1) * 128]
                                        .unsqueeze(1)
                                        .broadcast_to([64, 2, 128])
                                    )
                                    rhs = (
                                        qt_sb[a * 64 : (a + 1) * 64, j * 512 : (j + 1) * 512]
                                        .unsqueeze(1)
                                        .broadcast_to([64, 2, 512])
                                    )
                                    nc.tensor.matmul(
                                        sc[:, ti, :], lhs, rhs,
                                        start=True, stop=True, perf_mode=DR,
                                    )
                                pt = ptp.tile(
                                    [128, 2, 512], BF16, tag="pt", name=f"pt{p}_{a}_{j}_{c}"
                                )
                                nc.scalar.activation(
                                    pt[:].rearrange("p a b -> p (a b)"),
                                    sc[:].rearrange("p a b -> p (a b)"),
                                    AF.Exp,
                                    scale=EXP_SCALE,
                                )
                                pts.append(pt)
                                if c == 1:
                                    flush_pend()  # previous unit's AV burst
                                slot += 1
                                while taken * 32 < n * slot and taken < n:
                                    closures[taken]()
                                    taken += 1
                            pend[0] = (p, a, j, av, pts)
                    while taken < n:
                        closures[taken]()
                        taken += 1

                # ---------------- lead-in ----------------
                # weight DMAs first so the first K chunk isn't stuck behind
                # the full 8 MiB x load on the DMA queues
                dma_wkq(0)
                dma_x(0)
                dma_wv(0)
                for j in range(1, 4):
                    dma_x(j)
                lead_kq = kq_closures(0)
                for f in lead_kq:
                    f()
                # all of duo 0's V must be emitted before the first AV burst
                # (reads emitted before their writes see stale memory)
                for f in v_closures(0):
                    f()

                # ---------------- pair loop ----------------
                for p in range(NPAIR):
                    stream = []
                    if p + 1 < NPAIR:
                        dma_wkq(p + 1)
                    if p % 2 == 0 and p + 2 < NPAIR:
                        dma_wv(p // 2 + 1)
                    if p % 2 == 1 and p + 1 < NPAIR:
                        stream += v_closures(p // 2 + 1)
                    if p + 1 < NPAIR:
                        stream += kq_closures(p + 1)
                    if p == 5:
                        nc.sync.dma_start(
                            wo_sb[:], wo.ap().rearrange("(k r) c -> r k c", r=128)
                        )
                    attn_pair(p, stream)
                flush_pend()

                if dbg:
                    with tc.tile_pool(name="dbgp", bufs=2) as dbgp:
                        def dump(dst_ap, src_ap, n, w):
                            for i in range(n):
                                dt_ = dbgp.tile([128, w], F32, tag="dbg", name=f"dbg{i}")
                                nc.vector.tensor_copy(dt_[:], src_ap(i))
                                nc.sync.dma_start(dst_ap(i), dt_[:])
                        dump(lambda i: dbg_kt.ap()[:, i*1024:(i+1)*1024],
                             lambda i: kt_t[7][:, i*1024:(i+1)*1024], 2, 1024)
                        dump(lambda i: dbg_qt.ap()[:, :],
                             lambda i: qt_t[7][:, :], 1, 1024)
                        dump(lambda i: dbg_vt.ap()[:, 4*i:4*(i+1), :].rearrange("p t c -> p (t c)"),
                             lambda i: vt_t[3][:, 4*i:4*(i+1), :].rearrange("p t c -> p (t c)"), 4, 1040)
                        dump(lambda i: dbg_ot.ap()[:, i, :],
                             lambda i: outT[:, i, :], 8, 1024)

            # ---------------- output projection ----------------
            with (
                tc.tile_pool(name="yps", bufs=_B("YPS", 6), space="PSUM") as ypsp,
                tc.tile_pool(name="yd", bufs=_B("YD", 4)) as ydp,
            ):
                for m in range(SQ // 128):
                    yps = [
                        ypsp.tile([128, 512], F32, tag="yps", name=f"yp{m}_{nb}")
                        for nb in range(2)
                    ]
                    for p in range(NPAIR):
                        for nb in range(2):
                            nc.tensor.matmul(
                                yps[nb][:],
                                outT[:, p, m * 128 : (m + 1) * 128],
                                wo_sb[:, p, nb * 512 : (nb + 1) * 512],
                                start=(p == 0),
                                stop=False,
                            )
                    for nb in range(2):
                        nc.tensor.matmul(
                            yps[nb][:],
                            ones_sb[:],
                            bo_sb[:, nb * 512 : (nb + 1) * 512],
                            start=False,
                            stop=True,
                        )
                        ysb = ydp.tile([128, 512], F32, tag="ysb", name=f"ysb{m}_{nb}")
                        nc.vector.tensor_copy(ysb[:], yps[nb][:])
                        nc.sync.dma_start(
                            y.ap()[m * 128 : (m + 1) * 128, nb * 512 : (nb + 1) * 512],
                            ysb[:],
                        )

    nc.compile()
    return nc


def prep_inputs(x, Wq, bq, Wk, bk, Wv, bv, Wo, bo):
    """Host-side sharding: returns per-core input maps (numpy only)."""
    import ml_dtypes

    x = np.asarray(x, dtype=np.float32)
    Wq = np.asarray(Wq, dtype=np.float32)
    Wk = np.asarray(Wk, dtype=np.float32)
    Wv = np.asarray(Wv, dtype=np.float32)
    Wo = np.asarray(Wo, dtype=np.float32)
    bq = np.asarray(bq, dtype=np.float32)
    bk = np.asarray(bk, dtype=np.float32)
    bv = np.asarray(bv, dtype=np.float32)
    bo = np.asarray(bo, dtype=np.float32)

    shared = {
        "wq": np.ascontiguousarray(8.0 * Wq.transpose(1, 0, 2).reshape(D_MODEL, D_MODEL)),
        "wk": np.ascontiguousarray(8.0 * Wk.transpose(1, 0, 2).reshape(D_MODEL, D_MODEL)),
        "wv": np.ascontiguousarray(Wv.transpose(1, 0, 2).reshape(D_MODEL, D_MODEL)),
        "wo": np.ascontiguousarray(Wo.T).astype(ml_dtypes.bfloat16),
        "bq": np.ascontiguousarray((8.0 * bq).reshape(NPAIR, 128).T),
        "bk": np.ascontiguousarray((8.0 * bk).reshape(NPAIR, 128).T),
        "bv": np.ascontiguousarray(bv.reshape(NPAIR, 128).T),
        "bo": bo.reshape(1, D_MODEL).copy(),
        "ones_in": np.ones((1, 128), dtype=np.float32),
        "ident_in": np.eye(128, dtype=ml_dtypes.bfloat16),
    }
    in_maps = []
    for core in range(N_CORES):
        b, half = divmod(core, 2)
        xt = x[b].T
        if half == 0:
            xt_core = xt
        else:
            xt_core = np.concatenate([xt[:, SQ:], xt[:, :SQ]], axis=1)
        in_maps.append({"xT": np.ascontiguousarray(xt_core), **shared})
    return in_maps


def assemble_output(results):
    y = np.empty((B, S, D_MODEL), dtype=np.float32)
    for core in range(N_CORES):
        b, half = divmod(core, 2)
        y[b, half * SQ : (half + 1) * SQ, :] = results[core]["y"]
    return y


def _get_runner():
    """Build the program + jitted 8-core executor once; reuse across calls."""
    if "runner" in _CACHE:
        return _CACHE["runner"]

    import jax
    import concourse.mybir as mb
    from concourse import bass2jax
    from jax.sharding import Mesh, PartitionSpec
    from jax.experimental.shard_map import shard_map

    nc = build_program()
    _CACHE["nc"] = nc
    bass2jax.install_neuronx_cc_hook()

    partition_name = (
        nc.partition_id_tensor.name if nc.partition_id_tensor is not None else None
    )
    in_names, out_names, out_avals = [], [], []
    for alloc in nc.m.functions[0].allocations:
        if not isinstance(alloc, mb.MemoryLocationSet):
            continue
        name = alloc.memorylocations[0].name
        if alloc.kind == "ExternalInput":
            if name != partition_name:
                in_names.append(name)
        elif alloc.kind == "ExternalOutput":
            out_names.append(name)
            out_avals.append(
                jax.core.ShapedArray(tuple(alloc.tensor_shape), mb.dt.np(alloc.dtype))
            )
    n_params = len(in_names)
    n_outs = len(out_avals)
    all_in_names = in_names + out_names
    if partition_name is not None:
        all_in_names = all_in_names + [partition_name]

    def _body(*args):
        operands = list(args)
        if partition_name is not None:
            operands.append(bass2jax.partition_id_tensor())
        outs = bass2jax._bass_exec_p.bind(
            *operands,
            out_avals=tuple(out_avals),
            in_names=tuple(all_in_names),
            out_names=tuple(out_names),
            lowering_input_output_aliases=(),
            sim_require_finite=True,
            sim_require_nnan=True,
            nc=nc,
        )
        return tuple(outs)

    devices = jax.devices()[:N_CORES]
    mesh = Mesh(np.asarray(devices), ("core",))
    donate = tuple(range(n_params, n_params + n_outs))
    sharded = jax.jit(
        shard_map(
            _body,
            mesh=mesh,
            in_specs=(PartitionSpec("core"),) * (n_params + n_outs),
            out_specs=(PartitionSpec("core"),) * n_outs,
            check_rep=False,
        ),
        donate_argnums=donate,
        keep_unused=True,
    )

    import hashlib

    from jax.sharding import NamedSharding

    sharding = NamedSharding(mesh, PartitionSpec("core"))
    dev_cache: dict = {}

    # donated output buffers are created on-device (no host->device transfer)
    import jax.numpy as jnp

    zeros_fns = [
        jax.jit(
            (lambda shape, dtype: (lambda: jnp.zeros(shape, dtype)))(
                (N_CORES * a.shape[0], *a.shape[1:]), a.dtype
            ),
            out_shardings=sharding,
        )
        for a in out_avals
    ]

    def _dev_input(nm, in_maps):
        arrs = [np.asarray(m[nm]) for m in in_maps]
        h = hashlib.blake2b(digest_size=16)
        for a in arrs:
            h.update(a.tobytes())
        key = (nm, h.hexdigest())
        if key not in dev_cache:
            if len(dev_cache) > 64:
                dev_cache.clear()
            dev_cache[key] = jax.device_put(
                np.concatenate(arrs, axis=0), sharding
            )
        return dev_cache[key]

    def run(in_maps):
        concat_in = [_dev_input(nm, in_maps) for nm in in_names]
        concat_zeros = [zf() for zf in zeros_fns]
        out_arrs = sharded(*concat_in, *concat_zeros)
        return [
            {
                nm: np.asarray(out_arrs[i]).reshape(N_CORES, *out_avals[i].shape)[c]
                for i, nm in enumerate(out_names)
            }
            for c in range(N_CORES)
        ]

    _CACHE["runner"] = run
    return run


def kernel(**inputs):
    run = _get_runner()
    in_maps = prep_inputs(**inputs)
    return assemble_output(run(in_maps))


# revision 23
# speedup vs baseline: 1.5293x; 1.0999x over previous
"""Multi-head attention kernel for Trainium2, 8 NeuronCores.

Sharding: data-parallel over (batch, query-half): core i handles batch i//2
and query rows (i%2)*1024 ... +1024. Each core computes K/V over the full
sequence of its batch, Q over its query half, attention for all 16 heads,
and the output projection for its query rows. No collectives.

Fully fused, SBUF-resident pipeline (no DRAM scratch):
  per head-pair p (2 heads):
    K^T = (8*Wk_p)^T x^T + 8bk -> fp8 e4m3 [128, 2048]   (PE fp32r + DVE drain)
    Q^T = (8*Wq_p)^T x^T + 8bq -> fp8 [128, 1024]
    V   = x Wv_duo             -> bf16 [128 sk, 16t, 260] (pair-duo, ones cols)
    scores^T[sk,sq] = 2*K^T_slice.T Q^T  via fp8 DoubleRow matmul (both
        operands stride-0-doubled; x2 folded into the exp scale) -> PSUM
    P^T = exp(scores/1024) -> bf16 (ACT, 1024-col chunks)
    AV flipped: out[sq,65] = sum_t P^T-tile.T @ [V|1]  (bf16, all 128 output
        partitions used; col 64 = softmax denominator)
    normalize on DVE (per-partition reciprocal; no cross-partition broadcast)
    PE-transpose out -> outT[d, sq] bf16 (+bv bias on the DVE drain)
  y = outT^T Wo^T + bo (fp32r moving, bf16 stationary)

Engine budget per core (cost model): PE ~284us, ACT exp ~266us, DVE ~165us,
DMA ~28 MiB. fp8 scores rel-err ~9e-3 vs fp32 reference (2e-2 budget).
"""

import os

os.environ.setdefault("MYCRO_LOCAL_CACHE", "1")

import numpy as np

_B = lambda k, d: int(os.environ.get(k, d))

try:
    import concourse.bass as bass
except ImportError:  # pragma: no cover
    import sys

    for p in ("/opt/trn_rl_repo", "/root/.axon_site/_ro/trn_rl_repo"):
        if os.path.isdir(p) and p not in sys.path:
            sys.path.insert(0, p)
    import concourse.bass as bass

import concourse.mybir as mybir
import concourse.tile as tile
from concourse import bacc, bass_utils

F32R = mybir.dt.float32r
F32 = mybir.dt.float32
BF16 = mybir.dt.bfloat16
FP8 = mybir.dt.float8e4
AF = mybir.ActivationFunctionType
DR = mybir.MatmulPerfMode.DoubleRow

B = 4
S = 2048
D_MODEL = 1024
H = 16
HD = 64
NPAIR = 8
KT = 8
SQ = 1024
NT = S // 128  # 16 sk-tiles
N_CORES = 8
EXP_SCALE = 1.0 / 1024.0  # 1/8 softmax scale / (8*8 fp8 scales) / 2 (stride-0 DR)

_CACHE: dict = {}


def build_program():
    nc = bacc.Bacc("TRN2", target_bir_lowering=False, debug=False)

    xT = nc.dram_tensor("xT", [D_MODEL, S], BF16, kind="ExternalInput")
    wq = nc.dram_tensor("wq", [D_MODEL, D_MODEL], BF16, kind="ExternalInput")
    wk = nc.dram_tensor("wk", [D_MODEL, D_MODEL], BF16, kind="ExternalInput")
    wv = nc.dram_tensor("wv", [D_MODEL, D_MODEL], BF16, kind="ExternalInput")
    wo = nc.dram_tensor("wo", [D_MODEL, D_MODEL], BF16, kind="ExternalInput")
    bq = nc.dram_tensor("bq", [128, NPAIR], F32, kind="ExternalInput")
    bk = nc.dram_tensor("bk", [128, NPAIR], F32, kind="ExternalInput")
    bv = nc.dram_tensor("bv", [128, NPAIR], F32, kind="ExternalInput")
    bo = nc.dram_tensor("bo", [1, D_MODEL], F32R, kind="ExternalInput")
    ones_in = nc.dram_tensor("ones_in", [1, 128], F32R, kind="ExternalInput")
    ident_in = nc.dram_tensor("ident_in", [128, 128], BF16, kind="ExternalInput")
    y = nc.dram_tensor("y", [SQ, D_MODEL], F32, kind="ExternalOutput")
    dbg = os.environ.get("KDBG", "0") == "1"
    if dbg:
        dbg_kt = nc.dram_tensor("dbg_kt", [128, S], F32, kind="ExternalOutput")
        dbg_qt = nc.dram_tensor("dbg_qt", [128, SQ], F32, kind="ExternalOutput")
        dbg_vt = nc.dram_tensor("dbg_vt", [128, NT, 260], F32, kind="ExternalOutput")
        dbg_ot = nc.dram_tensor("dbg_ot", [128, NPAIR, SQ], F32, kind="ExternalOutput")

    with tile.TileContext(nc) as tc:
        with tc.tile_pool(name="pers", bufs=1) as pers:
            xt_sb = pers.tile([128, KT, S], BF16)
            outT = pers.tile([128, NPAIR, SQ], BF16)
            wo_sb = pers.tile([128, KT, D_MODEL], BF16)
            ident_sb = pers.tile([128, 128], BF16)
            ones_sb = pers.tile([1, 128], F32R)
            bq_sb = pers.tile([128, NPAIR], F32)
            bk_sb = pers.tile([128, NPAIR], F32)
            bv_sb = pers.tile([128, NPAIR], F32)
            bo_sb = pers.tile([1, D_MODEL], F32R)
            y6_sb = pers.tile([128, SQ // 128, D_MODEL], BF16)
            def dma_small():
                nc.sync.dma_start(bk_sb[:], bk.ap())
                nc.sync.dma_start(bq_sb[:], bq.ap())
                nc.sync.dma_start(ident_sb[:], ident_in.ap())
                nc.sync.dma_start(ones_sb[:], ones_in.ap())
                nc.sync.dma_start(bv_sb[:], bv.ap())
                nc.sync.dma_start(bo_sb[:], bo.ap())

            def dma_x(j, w=256):
                # coalesced gather: one dma_start per 256-col block (1 MiB)
                for c0 in range(j * 512, (j + 1) * 512, w):
                    nc.sync.dma_start(
                        xt_sb[:, :, c0 : c0 + w],
                        xT.ap()[:, c0 : c0 + w].rearrange("(k r) c -> r k c", r=128),
                    )

            with (
                tc.tile_pool(name="wkq", bufs=2) as wkqp,
                tc.tile_pool(name="wvd", bufs=2) as wvdp,
                tc.tile_pool(name="ktp", bufs=2) as ktp,
                tc.tile_pool(name="qtp", bufs=2) as qtp,
                tc.tile_pool(name="vtp", bufs=2) as vtp,
                tc.tile_pool(name="ptp", bufs=2) as ptp,
                tc.tile_pool(name="ntp", bufs=2) as ntp,
                tc.tile_pool(name="scp", bufs=_B("SCP", 2), space="PSUM") as scp,
                tc.tile_pool(name="ppp", bufs=_B("PPP", 2), space="PSUM") as ppp,
                tc.tile_pool(name="avt", bufs=_B("AVT", 2), space="PSUM") as avtp,
            ):
                wk_t, wq_t, wv_t = {}, {}, {}
                kt_t, qt_t, vt_t = {}, {}, {}

                def pp_chunk(name):
                    return ppp.tile([128, 256], F32, tag="pp", name=name)

                def dma_wkq(p):
                    wk_sb = wkqp.tile([128, KT, 128], BF16, tag="wk", name=f"wk{p}")
                    nc.sync.dma_start(
                        wk_sb[:],
                        wk.ap()[:, 128 * p : 128 * (p + 1)].rearrange(
                            "(k r) c -> r k c", r=128
                        ),
                    )
                    wk_t[p] = wk_sb
                    wq_sb = wkqp.tile([128, KT, 128], BF16, tag="wq", name=f"wq{p}")
                    nc.sync.dma_start(
                        wq_sb[:],
                        wq.ap()[:, 128 * p : 128 * (p + 1)].rearrange(
                            "(k r) c -> r k c", r=128
                        ),
                    )
                    wq_t[p] = wq_sb

                def dma_wv(d):
                    wv_sb = wvdp.tile([128, KT, 256], BF16, tag="wv", name=f"wv{d}")
                    nc.sync.dma_start(
                        wv_sb[:],
                        wv.ap()[:, 256 * d : 256 * (d + 1)].rearrange(
                            "(k r) c -> r k c", r=128
                        ),
                    )
                    wv_t[d] = wv_sb

                def kq_closures(p):
                    """K (8) + Q (4) psum-chunk closures for pair p."""
                    kt_sb = ktp.tile([128, S], FP8, tag="kt", name=f"kt{p}")
                    qt_sb = qtp.tile([128, SQ], FP8, tag="qt", name=f"qt{p}")
                    kt_t[p], qt_t[p] = kt_sb, qt_sb
                    cls = []

                    def k_chunk(blk):
                        def f():
                            ps = pp_chunk(f"kp{p}_{blk}")
                            for kc in range(KT):
                                nc.tensor.matmul(
                                    ps[:],
                                    wk_t[p][:, kc, :],
                                    xt_sb[:, kc, blk * 256 : (blk + 1) * 256],
                                    start=(kc == 0),
                                    stop=(kc == KT - 1),
                                )
                            nc.vector.tensor_scalar_add(
                                kt_sb[:, blk * 256 : (blk + 1) * 256],
                                ps[:],
                                bk_sb[:, p : p + 1],
                            )
                        return f

                    def q_chunk(blk):
                        def f():
                            ps = pp_chunk(f"qp{p}_{blk}")
                            for kc in range(KT):
                                nc.tensor.matmul(
                                    ps[:],
                                    wq_t[p][:, kc, :],
                                    xt_sb[:, kc, blk * 256 : (blk + 1) * 256],
                                    start=(kc == 0),
                                    stop=(kc == KT - 1),
                                )
                            nc.vector.tensor_scalar_add(
                                qt_sb[:, blk * 256 : (blk + 1) * 256],
                                ps[:],
                                bq_sb[:, p : p + 1],
                            )
                        return f

                    # K first (scores of (a=0, j=0) touch all sk), Q interleaved
                    for blk in range(8):
                        cls.append(k_chunk(blk))
                        if blk < 4:
                            cls.append(q_chunk(blk))
                    return cls

                def kq_closures_split(p):
                    """Lead variant: minimal immediate prefix + deferred rest.

                    sc chunk c of unit (a, j=0) reads kt cols c*256 (= K chunk
                    c) and qt blk 0-1, so only K0, K1, Q0, Q1 must precede the
                    first score matmuls; K2..K7 stream 1:1 ahead of sc chunks.
                    """
                    cls = kq_closures(p)
                    # cls order: K0 Q0 K1 Q1 K2 Q2 K3 Q3 K4 K5 K6 K7
                    imm = [cls[i] for i in (0, 2, 1, 3)]
                    tail = [cls[i] for i in (4, 6, 8, 9, 10, 11, 5, 7)]
                    return imm, tail

                def v_closures(d):
                    """16 V psum-chunk closures for pair-duo d (pairs 2d, 2d+1)."""
                    vt_sb = vtp.tile([128, NT, 260], BF16, tag="vt", name=f"vt{d}")
                    vt_t[d] = vt_sb
                    cls = []

                    def ones_cols():
                        nc.vector.memset(
                            vt_sb[:].rearrange("p t (c f) -> p t c f", f=65)[
                                :, :, :, 64:65
                            ],
                            1.0,
                        )

                    cls.append(ones_cols)

                    def v_chunk(t):
                        def f():
                            ps = pp_chunk(f"vp{d}_{t}")
                            for kc in range(KT):
                                nc.tensor.matmul(
                                    ps[:],
                                    xt_sb[:, kc, t * 128 : (t + 1) * 128],
                                    wv_t[d][:, kc, :],
                                    start=(kc == 0),
                                    stop=(kc == KT - 1),
                                )
                            nc.vector.tensor_copy(
                                vt_sb[:, t, :].rearrange("p (c f) -> p c f", f=65)[
                                    :, :, 0:64
                                ],
                                ps[:].rearrange("p (c f) -> p c f", f=64),
                            )
                        return f

                    for t in range(NT):
                        cls.append(v_chunk(t))
                    return cls

                # -------------------- attention pipeline --------------------
                # AV is emitted as a per-unit burst with each ms-group's
                # accumulation sequential: PSUM start_tensor_calc pends the
                # whole 2KB bank, so groups sharing a bank must not interleave.
                pend = [None]

                def finish_unit(p, a, j, av):
                    rc = ntp.tile([128, 4], F32, tag="rc", name=f"rc{p}_{a}_{j}")
                    nc.vector.reciprocal(rc[:], av[:, :, 64])
                    nt = ntp.tile([128, 4, 64], BF16, tag="nt", name=f"nt{p}_{a}_{j}")
                    nc.vector.tensor_mul(
                        nt[:], av[:, :, 0:64], rc[:].unsqueeze(2).broadcast_to([128, 4, 64])
                    )
                    tp = avtp.tile(
                        [128, 4, 128], BF16, tag="avtp", name=f"tp{p}_{a}_{j}"
                    )
                    for ms in range(4):
                        nc.tensor.transpose(
                            tp[a * 64 : (a + 1) * 64, ms, :], nt[:, ms, :], ident_sb[:]
                        )
                    nc.vector.tensor_scalar_add(
                        outT[a * 64 : (a + 1) * 64, p, j * 512 : (j + 1) * 512],
                        tp[a * 64 : (a + 1) * 64, :, :].rearrange("p c f -> p (c f)"),
                        bv_sb[a * 64 : (a + 1) * 64, p : p + 1],
                    )

                def flush_pend():
                    if pend[0] is None:
                        return
                    p_, a_, j_, av_, pts_ = pend[0]
                    pend[0] = None
                    pin = p_ % 2
                    voff = pin * 130 + a_ * 65
                    vt_sb = vt_t[p_ // 2]
                    for ms in range(4):
                        t = 0
                        for pt_, tc_ in pts_:
                            for ti in range(tc_):
                                nc.tensor.matmul(
                                    av_[:, ms, :],
                                    pt_[:, ti, ms * 128 : (ms + 1) * 128],
                                    vt_sb[:, t, voff : voff + 65],
                                    start=(t == 0),
                                    stop=(t == NT - 1),
                                )
                                t += 1
                    finish_unit(p_, a_, j_, av_)

                CHUNKS = (2,) * 8  # t-counts per exp chunk (sum 16)
                NSLOT = len(CHUNKS) * 4

                def attn_pair(p, closures, urgent=()):
                    n, taken, slot = len(closures), 0, 0
                    nu, ntaken = len(urgent), 0
                    kt_sb, qt_sb = kt_t[p], qt_t[p]
                    for a in range(2):
                        for j in range(2):
                            av = avtp.tile(
                                [128, 4, 65], F32, tag="avtp", name=f"av{p}_{a}_{j}"
                            )
                            pts = []
                            t = 0
                            for c, tc_ in enumerate(CHUNKS):
                                sc = scp.tile(
                                    [128, tc_, 512], F32, tag=f"sc{tc_}", bufs=2,
                                    name=f"sc{p}_{a}_{j}_{c}",
                                )
                                for ti in range(tc_):
                                    lhs = (
                                        kt_sb[a * 64 : (a + 1) * 64, t * 128 : (t + 1) * 128]
                                        .unsqueeze(1)
                                        .broadcast_to([64, 2, 128])
                                    )
                                    rhs = (
                                        qt_sb[a * 64 : (a + 1) * 64, j * 512 : (j + 1) * 512]
                                        .unsqueeze(1)
                                        .broadcast_to([64, 2, 512])
                                    )
                                    nc.tensor.matmul(
                                        sc[:, ti, :], lhs, rhs,
                                        start=True, stop=True, perf_mode=DR,
                                    )
                                    t += 1
                                pt = ptp.tile(
                                    [128, tc_, 512], BF16, tag=f"pt{tc_}",
                                    bufs=_B("PTB", 17),
                                    name=f"pt{p}_{a}_{j}_{c}",
                                )
                                nc.scalar.activation(
                                    pt[:].rearrange("p a b -> p (a b)"),
                                    sc[:].rearrange("p a b -> p (a b)"),
                                    AF.Exp,
                                    scale=EXP_SCALE,
                                )
                                pts.append((pt, tc_))
                                if c == len(CHUNKS) - 1:
                                    flush_pend()  # previous unit's AV burst
                                slot += 1
                                nun = 2 * len(CHUNKS) - 1
                                while ntaken * nun < nu * min(slot, nun) and ntaken < nu:
                                    urgent[ntaken]()
                                    ntaken += 1
                                while taken * (NSLOT - nun) < n * max(slot - nun, 0) and taken < n:
                                    closures[taken]()
                                    taken += 1
                            pend[0] = (p, a, j, av, pts)
                    while ntaken < nu:
                        urgent[ntaken]()
                        ntaken += 1
                    while taken < n:
                        closures[taken]()
                        taken += 1

                def y6_closures():
                    cls = []

                    def y6_chunk(m, nb):
                        def f():
                            ps = ppp.tile([128, 512], F32, tag="pp", name=f"y6_{m}_{nb}")
                            for p6 in range(6):
                                nc.tensor.matmul(
                                    ps[:],
                                    outT[:, p6, m * 128 : (m + 1) * 128],
                                    wo_sb[:, p6, nb * 512 : (nb + 1) * 512],
                                    start=(p6 == 0),
                                    stop=(p6 == 5),
                                )
                            nc.vector.tensor_copy(
                                y6_sb[:, m, nb * 512 : (nb + 1) * 512], ps[:]
                            )
                        return f

                    for m in range(SQ // 128):
                        for nb in range(2):
                            cls.append(y6_chunk(m, nb))
                    return cls

                # ---------------- lead-in ----------------
                # weight DMAs first so the first K chunk isn't stuck behind
                # the full 8 MiB x load on the DMA queues
                dma_wkq(0)
                dma_x(0)
                dma_small()
                dma_wv(0)
                for j in range(1, 4):
                    dma_x(j)
                lead_imm, lead_tail = kq_closures_split(0)
                for f in lead_imm:
                    f()
                # v(0) + Q(0) tail stream through attn(0)'s first unit (they
                # must all be emitted before the first AV burst at unit 1)
                urgent0 = lead_tail + v_closures(0)

                # ---------------- pair loop ----------------
                for p in range(NPAIR):
                    stream = []
                    if p + 1 < NPAIR:
                        dma_wkq(p + 1)
                    if p % 2 == 0 and p + 2 < NPAIR:
                        dma_wv(p // 2 + 1)
                    if p % 2 == 1 and p + 1 < NPAIR:
                        stream += v_closures(p // 2 + 1)
                    if p + 1 < NPAIR:
                        stream += kq_closures(p + 1)
                    if p == 4:
                        nc.sync.dma_start(
                            wo_sb[:], wo.ap().rearrange("(k r) c -> r k c", r=128)
                        )
                    if p == 6:
                        stream += y6_closures()[:8]
                    if p == 7:
                        stream += y6_closures()[8:]
                    attn_pair(p, stream, urgent=urgent0 if p == 0 else ())
                flush_pend()

                if dbg:
                    with tc.tile_pool(name="dbgp", bufs=2) as dbgp:
                        def dump(dst_ap, src_ap, n, w):
                            for i in range(n):
                                dt_ = dbgp.tile([128, w], F32, tag="dbg", name=f"dbg{i}")
                                nc.vector.tensor_copy(dt_[:], src_ap(i))
                                nc.sync.dma_start(dst_ap(i), dt_[:])
                        dump(lambda i: dbg_kt.ap()[:, i*1024:(i+1)*1024],
                             lambda i: kt_t[7][:, i*1024:(i+1)*1024], 2, 1024)
                        dump(lambda i: dbg_qt.ap()[:, :],
                             lambda i: qt_t[7][:, :], 1, 1024)
                        dump(lambda i: dbg_vt.ap()[:, 4*i:4*(i+1), :].rearrange("p t c -> p (t c)"),
                             lambda i: vt_t[3][:, 4*i:4*(i+1), :].rearrange("p t c -> p (t c)"), 4, 1040)
                        dump(lambda i: dbg_ot.ap()[:, i, :],
                             lambda i: outT[:, i, :], 8, 1024)

            # ---------------- output projection ----------------
            with (
                tc.tile_pool(name="yps", bufs=_B("YPS", 6), space="PSUM") as ypsp,
                tc.tile_pool(name="yd", bufs=_B("YD", 4)) as ydp,
            ):
                for m in range(SQ // 128):
                    yps = [
                        ypsp.tile([128, 512], F32, tag="yps", name=f"yp{m}_{nb}")
                        for nb in range(2)
                    ]
                    for p in (6, 7):
                        for nb in range(2):
                            nc.tensor.matmul(
                                yps[nb][:],
                                outT[:, p, m * 128 : (m + 1) * 128],
                                wo_sb[:, p, nb * 512 : (nb + 1) * 512],
                                start=(p == 6),
                                stop=False,
                            )
                    for nb in range(2):
                        nc.tensor.matmul(
                            yps[nb][:],
                            ones_sb[:],
                            bo_sb[:, nb * 512 : (nb + 1) * 512],
                            start=False,
                            stop=True,
                        )
                        ysb = ydp.tile([128, 512], F32, tag="ysb", name=f"ysb{m}_{nb}")
                        nc.vector.tensor_add(
                            ysb[:], yps[nb][:], y6_sb[:, m, nb * 512 : (nb + 1) * 512]
                        )
                        nc.sync.dma_start(
                            y.ap()[m * 128 : (m + 1) * 128, nb * 512 : (nb + 1) * 512],
                            ysb[:],
                        )

    nc.compile()
    return nc


def prep_inputs(x, Wq, bq, Wk, bk, Wv, bv, Wo, bo):
    """Host-side sharding: returns per-core input maps (numpy only)."""
    import ml_dtypes

    x = np.asarray(x, dtype=np.float32)
    Wq = np.asarray(Wq, dtype=np.float32)
    Wk = np.asarray(Wk, dtype=np.float32)
    Wv = np.asarray(Wv, dtype=np.float32)
    Wo = np.asarray(Wo, dtype=np.float32)
    bq = np.asarray(bq, dtype=np.float32)
    bk = np.asarray(bk, dtype=np.float32)
    bv = np.asarray(bv, dtype=np.float32)
    bo = np.asarray(bo, dtype=np.float32)

    shared = {
        "wq": np.ascontiguousarray(8.0 * Wq.transpose(1, 0, 2).reshape(D_MODEL, D_MODEL)).astype(ml_dtypes.bfloat16),
        "wk": np.ascontiguousarray(8.0 * Wk.transpose(1, 0, 2).reshape(D_MODEL, D_MODEL)).astype(ml_dtypes.bfloat16),
        "wv": np.ascontiguousarray(Wv.transpose(1, 0, 2).reshape(D_MODEL, D_MODEL)).astype(ml_dtypes.bfloat16),
        "wo": np.ascontiguousarray(Wo.T).astype(ml_dtypes.bfloat16),
        "bq": np.ascontiguousarray((8.0 * bq).reshape(NPAIR, 128).T),
        "bk": np.ascontiguousarray((8.0 * bk).reshape(NPAIR, 128).T),
        "bv": np.ascontiguousarray(bv.reshape(NPAIR, 128).T),
        "bo": bo.reshape(1, D_MODEL).copy(),
        "ones_in": np.ones((1, 128), dtype=np.float32),
        "ident_in": np.eye(128, dtype=ml_dtypes.bfloat16),
    }
    in_maps = []
    for core in range(N_CORES):
        b, half = divmod(core, 2)
        xt = x[b].T
        if half == 0:
            xt_core = xt
        else:
            xt_core = np.concatenate([xt[:, SQ:], xt[:, :SQ]], axis=1)
        in_maps.append({"xT": np.ascontiguousarray(xt_core).astype(ml_dtypes.bfloat16), **shared})
    return in_maps


def assemble_output(results):
    y = np.empty((B, S, D_MODEL), dtype=np.float32)
    for core in range(N_CORES):
        b, half = divmod(core, 2)
        y[b, half * SQ : (half + 1) * SQ, :] = results[core]["y"]
    return y


def _get_runner():
    """Build the program + jitted 8-core executor once; reuse across calls."""
    if "runner" in _CACHE:
        return _CACHE["runner"]

    import jax
    import concourse.mybir as mb
    from concourse import bass2jax
    from jax.sharding import Mesh, PartitionSpec
    from jax.experimental.shard_map import shard_map

    nc = build_program()
    _CACHE["nc"] = nc
    bass2jax.install_neuronx_cc_hook()

    partition_name = (
        nc.partition_id_tensor.name if nc.partition_id_tensor is not None else None
    )
    in_names, out_names, out_avals = [], [], []
    for alloc in nc.m.functions[0].allocations:
        if not isinstance(alloc, mb.MemoryLocationSet):
            continue
        name = alloc.memorylocations[0].name
        if alloc.kind == "ExternalInput":
            if name != partition_name:
                in_names.append(name)
        elif alloc.kind == "ExternalOutput":
            out_names.append(name)
            out_avals.append(
                jax.core.ShapedArray(tuple(alloc.tensor_shape), mb.dt.np(alloc.dtype))
            )
    n_params = len(in_names)
    n_outs = len(out_avals)
    all_in_names = in_names + out_names
    if partition_name is not None:
        all_in_names = all_in_names + [partition_name]

    def _body(*args):
        operands = list(args)
        if partition_name is not None:
            operands.append(bass2jax.partition_id_tensor())
        outs = bass2jax._bass_exec_p.bind(
            *operands,
            out_avals=tuple(out_avals),
            in_names=tuple(all_in_names),
            out_names=tuple(out_names),
            lowering_input_output_aliases=(),
            sim_require_finite=True,
            sim_require_nnan=True,
            nc=nc,
        )
        return tuple(outs)

    devices = jax.devices()[:N_CORES]
    mesh = Mesh(np.asarray(devices), ("core",))
    donate = tuple(range(n_params, n_params + n_outs))
    sharded = jax.jit(
        shard_map(
            _body,
            mesh=mesh,
            in_specs=(PartitionSpec("core"),) * (n_params + n_outs),
            out_specs=(PartitionSpec("core"),) * n_outs,
            check_rep=False,
        ),
        donate_argnums=donate,
        keep_unused=True,
    )

    import hashlib

    from jax.sharding import NamedSharding

    sharding = NamedSharding(mesh, PartitionSpec("core"))
    dev_cache: dict = {}

    # donated output buffers are created on-device (no host->device transfer)
    import jax.numpy as jnp

    zeros_fns = [
        jax.jit(
            (lambda shape, dtype: (lambda: jnp.zeros(shape, dtype)))(
                (N_CORES * a.shape[0], *a.shape[1:]), a.dtype
            ),
            out_shardings=sharding,
        )
        for a in out_avals
    ]

    def _dev_input(nm, in_maps):
        arrs = [np.asarray(m[nm]) for m in in_maps]
        h = hashlib.blake2b(digest_size=16)
        for a in arrs:
            h.update(a.tobytes())
        key = (nm, h.hexdigest())
        if key not in dev_cache:
            if len(dev_cache) > 64:
                dev_cache.clear()
            dev_cache[key] = jax.device_put(
                np.concatenate(arrs, axis=0), sharding
            )
        return dev_cache[key]

    def run(in_maps):
        concat_in = [_dev_input(nm, in_maps) for nm in in_names]
        concat_zeros = [zf() for zf in zeros_fns]
        out_arrs = sharded(*concat_in, *concat_zeros)
        return [
            {
                nm: np.asarray(out_arrs[i]).reshape(N_CORES, *out_avals[i].shape)[c]
                for i, nm in enumerate(out_names)
            }
            for c in range(N_CORES)
        ]

    _CACHE["runner"] = run
    return run


def kernel(**inputs):
    run = _get_runner()
    in_maps = prep_inputs(**inputs)
    return assemble_output(run(in_maps))
